# revision 30
# baseline (speedup 1.0000x reference)
"""Fused attention kernel for Trainium2 (Bass/Tile), SPMD over 8 NeuronCores.

Problem: B=4, D=64, S=4096 fp32 attention
    A = softmax_k(K^T Q / sqrt(D));  R = V A;  out = concat(R, Q) on channel dim.

Sharding: 8 cores = 4 batches x 2 query-halves (Sq=2048 per core).

Final design, 73.4-74.8us HW (v1 baseline 84.3us; v2 row-tiled 76.4us):
  * S = K^T Q row-tiled 2x: D=64 contraction only fills half the PE array, so
    two k-tiles run CONCURRENTLY as row-tiles (tile_position (0,0)/(64,0))
    sharing the one moving Q stream (Q duplicated on partitions 64:128).
    K is plain fp16 (no hi/lo split).
  * R = V E col-tiled 2x: M=64 output channels only fill half the PE columns,
    so two k-tiles' R-partials run concurrently at col positions 0/64,
    accumulating into separate partition halves of one PSUM bank; the halves
    are summed on the host.
  * Z = ones^T E col-tiled 4x: four M=1 matmuls at col positions 0/32/64/96
    per 512-cycle batch; the 4 partial rows are summed on the host.
  * exp split across two engines: ACT (LUT exp -> fp16 E) handles 9/16 of
    the k-tile pairs; DVE handles 7/16 via a one-instruction Schraudolph
    integer exp targeting BF16 (i16 = trunc(s*16*log2e + const); bf16's
    8-bit exponent means no underflow/subnormals, so no clamp is needed).
  * No on-device softmax divide: ship raw R-halves + Z rows, host divides.
  * PIPE_LAG decouples the in-order PE queue from exp latency; dummy warm-up
    matmuls hold the PE HAM clock gate at full rate through the DMA head.
"""

import sys

sys.path.insert(0, "/opt/trn_rl_repo")

import numpy as np  # noqa: E402
import ml_dtypes  # noqa: E402

B, D, S = 4, 64, 4096
NCORES = 8
SQ = S * B // NCORES  # 2048 queries per core
QT = 512              # q-tile width
KT = 128              # k-tile width
NQT = SQ // QT        # 4 q-tiles per core
NKT = S // KT         # 32 k-tiles
NPAIR = NKT // 2      # 16 k-tile pairs per q-tile
EXP_BIAS = -13.0      # exp(s/8 + EXP_BIAS): cancels in softmax, avoids fp16 inf
LOG2E = 1.4426950408889634
# DVE Schraudolph: u16 = trunc(s * A_SCH + B_SCH), bitcast bf16
#   = 2^((s/8 - 13)*log2e) * (1 + sawtooth(~3%))
C_MAGIC = -4.0
A_SCH = 128.0 * LOG2E / 8.0
B_SCH = 128.0 * (127.0 + EXP_BIAS * LOG2E) + C_MAGIC
# pairs owned by ACT (rest go to DVE); alternating keeps both engines busy
ACT_PAIRS = frozenset({0, 2, 4, 6, 8, 10, 12, 14, 15})
PIPE_LAG = 5          # pairs between S-matmul/exp and the consuming R-matmul
N_WARM = 48           # dummy matmuls to pre-warm the PE HAM clock gate

_nc_cache = None


def _build():
    global _nc_cache
    if _nc_cache is not None:
        return _nc_cache
    import concourse.tile as tile
    from concourse import bacc, mybir

    nc = bacc.Bacc(None, target_bir_lowering=False)
    f32 = mybir.dt.float32
    f16 = mybir.dt.float16
    bf16 = mybir.dt.bfloat16
    u16 = mybir.dt.uint16

    kst = nc.dram_tensor("kst", [2 * D, NPAIR * KT], f16, kind="ExternalInput")
    qrep = nc.dram_tensor("qrep", [2 * D, SQ], f16, kind="ExternalInput")
    # per-tile dtype: fp16 bits for ACT-owned tiles, bf16 bits for DVE-owned
    vtin = nc.dram_tensor("vtin", [KT, NKT * D], u16, kind="ExternalInput")
    # raw R-partial halves (even k-tiles on rows 0:64, odd on 64:128)
    out_r = nc.dram_tensor("out_r", [2 * D, SQ], f32, kind="ExternalOutput")
    # Z partial rows (from PSUM partitions 0/32/64/96); host sums + divides
    out_z = nc.dram_tensor("out_z", [4, SQ], f32, kind="ExternalOutput")

    with tile.TileContext(nc) as tc:
        with (
            tc.tile_pool(name="singles", bufs=1) as singles,
            tc.tile_pool(name="e16", bufs=7) as pe16,
            tc.tile_pool(name="eb", bufs=7) as peb,
            tc.tile_pool(name="ro", bufs=2) as pro,
            tc.tile_pool(name="zo", bufs=2) as pzo,
            tc.tile_pool(name="ps_s", bufs=3, space="PSUM") as ps_s,
            tc.tile_pool(name="ps_r", bufs=1, space="PSUM") as ps_r,
            tc.tile_pool(name="ps_z", bufs=1, space="PSUM") as ps_z,
        ):
            k_sb = singles.tile([2 * D, NPAIR * KT], f16)
            q_sb = singles.tile([2 * D, SQ], f16)
            vt_sb = singles.tile([KT, NKT * D], u16)
            bias_sb = singles.tile([KT, 1], f32)
            warm_sb = singles.tile([KT, KT], f16)
            ones16 = singles.tile([KT, 1], f16)
            onesb = singles.tile([KT, 1], bf16)
            nc.vector.memset(bias_sb, EXP_BIAS)
            nc.vector.memset(warm_sb, 0.0)
            nc.vector.memset(ones16, 1.0)
            nc.vector.memset(onesb, 1.0)

            from concourse.tile_rust import add_dep_helper

            # Wave 1 (~0.4MB): first work units. Wave 2: everything else,
            # held back so its SDMA traffic doesn't delay the pipeline head.
            nc.sync.dma_start(out=q_sb[:, :QT], in_=qrep[:, :QT])
            nc.sync.dma_start(out=k_sb[:, : 4 * KT], in_=kst[:, : 4 * KT])
            d_kh = nc.sync.dma_start(out=k_sb[:, 4 * KT : 8 * KT], in_=kst[:, 4 * KT : 8 * KT])
            nc.gpsimd.dma_start(out=vt_sb[:, : 8 * D], in_=vtin[:, : 8 * D])
            w2 = [
                nc.sync.dma_start(out=k_sb[:, 8 * KT :], in_=kst[:, 8 * KT :]),
                nc.gpsimd.dma_start(out=vt_sb[:, 8 * D :], in_=vtin[:, 8 * D :]),
                nc.sync.dma_start(out=q_sb[:, QT:], in_=qrep[:, QT:]),
            ]
            for bulk in w2:
                add_dep_helper(
                    bulk.ins, d_kh.ins, sync=True,
                    reason="bulk input DMA after first work unit",
                )

            vt = vt_sb.rearrange("p (j d) -> p j d", j=NKT)

            # HAM warm-up: PE activity with no DMA dependency so the clock
            # gate reaches 8/8 before the real matmuls arrive. Writes into the
            # first s_ps ring slot (WAW-serialized; real S-matmul clears it).
            first_s = None
            if N_WARM:
                first_s = ps_s.tile([KT, 2 * QT], f32, name="s_ps")
                for _ in range(N_WARM):
                    nc.tensor.matmul(
                        first_s[:, :KT], warm_sb, warm_sb, start=True, stop=True
                    )

            Exp = mybir.ActivationFunctionType.Exp
            mult = mybir.AluOpType.mult
            add = mybir.AluOpType.add

            r_ps_of = {}
            z_ps_of = {}
            pending = []

            def emit_R(t, g, rhs_pair, lhsTs, zops):
                r_ps = r_ps_of[t]
                z_ps = z_ps_of[t]
                # col-tiled R pair: k-tile 2g -> rows 0:64, 2g+1 -> 64:128
                nc.tensor.matmul(
                    r_ps[0:D, :], lhsTs[0], rhs_pair[0],
                    start=(g == 0), stop=(g == NPAIR - 1),
                    tile_position=(0, 0),
                )
                nc.tensor.matmul(
                    r_ps[D : 2 * D, :], lhsTs[1], rhs_pair[1],
                    start=(g == 0), stop=(g == NPAIR - 1),
                    tile_position=(0, D),
                )
                zops.append(rhs_pair)
                if g % 2 == 1:
                    # Z batch: 4 col-tiled M=1 matmuls over k-tiles 4b..4b+3
                    b = g // 2
                    for i in (0, 1, 2, 3):
                        rhs = zops[i // 2][i % 2]
                        ones = ones16 if rhs.dtype == f16 else onesb
                        nc.tensor.matmul(
                            z_ps[32 * i : 32 * i + 1, :], ones, rhs,
                            start=(b == 0), stop=(b == NPAIR // 2 - 1),
                            tile_position=(0, 32 * i),
                        )
                    zops.clear()
                if g == NPAIR - 1:
                    r_sb = pro.tile([2 * D, QT], f32, tag="r_sb")
                    nc.scalar.copy(out=r_sb, in_=r_ps)
                    nc.sync.dma_start(out=out_r[:, t * QT : (t + 1) * QT], in_=r_sb)
                    z_sb = pzo.tile([3 * 32 + 1, QT], f32, tag="z_sb")
                    nc.scalar.copy(out=z_sb, in_=z_ps[0 : 3 * 32 + 1, :])
                    for i in (0, 1, 2, 3):
                        nc.sync.dma_start(
                            out=out_z[i : i + 1, t * QT : (t + 1) * QT],
                            in_=z_sb[32 * i : 32 * i + 1, :],
                        )
                    del r_ps_of[t], z_ps_of[t]

            for t in range(NQT):
                r_ps_of[t] = ps_r.tile([2 * D, QT], f32, name="r_ps")
                z_ps_of[t] = ps_z.tile([KT, QT], f32, name="z_ps")
                zops = []
                for g in range(NPAIR):
                    if first_s is not None:
                        s_ps, first_s = first_s, None
                    else:
                        s_ps = ps_s.tile([KT, 2 * QT], f32, name="s_ps")
                    # row-tiled S pair: k-tile 2g on array rows 0:64,
                    # k-tile 2g+1 on rows 64:128, concurrent.
                    nc.tensor.matmul(
                        s_ps[:, :QT],
                        k_sb[0:D, g * KT : (g + 1) * KT],
                        q_sb[0:D, t * QT : (t + 1) * QT],
                        start=True,
                        stop=True,
                        tile_position=(0, 0),
                    )
                    nc.tensor.matmul(
                        s_ps[:, QT:],
                        k_sb[D : 2 * D, g * KT : (g + 1) * KT],
                        q_sb[D : 2 * D, t * QT : (t + 1) * QT],
                        start=True,
                        stop=True,
                        tile_position=(D, 0),
                    )
                    j0 = 2 * g
                    if g in ACT_PAIRS:
                        e16 = pe16.tile([KT, 2 * QT], f16, tag="e16")
                        nc.scalar.activation(
                            out=e16, in_=s_ps, func=Exp, scale=0.125, bias=bias_sb
                        )
                        rhs_pair = (e16[:, :QT], e16[:, QT:])
                        lhsTs = (
                            vt[:, j0, 0:D].bitcast(f16),
                            vt[:, j0 + 1, 0:D].bitcast(f16),
                        )
                    else:
                        ebu = peb.tile([KT, 2 * QT], u16, tag="eb")
                        nc.vector.tensor_scalar(
                            ebu, s_ps, A_SCH, B_SCH, mult, add
                        )
                        ebf = ebu.bitcast(bf16)
                        rhs_pair = (ebf[:, :QT], ebf[:, QT:])
                        lhsTs = (
                            vt[:, j0, 0:D].bitcast(bf16),
                            vt[:, j0 + 1, 0:D].bitcast(bf16),
                        )
                    pending.append((t, g, rhs_pair, lhsTs, zops))
                    if len(pending) > PIPE_LAG:
                        emit_R(*pending.pop(0))
            while pending:
                emit_R(*pending.pop(0))

    nc.compile()
    _nc_cache = nc
    return nc


def _in_maps(K, V, Q):
    K = np.asarray(K, dtype=np.float32)
    V = np.asarray(V, dtype=np.float32)
    Q = np.asarray(Q, dtype=np.float32)
    maps = []
    for c in range(NCORES):
        b, h = c // 2, c % 2
        # kst: pair-interleaved K tiles: rows 0:64 = even k-tiles, 64:128 = odd
        k16 = K[b].astype(np.float16).reshape(D, NKT, KT)
        kst = np.concatenate([k16[:, 0::2], k16[:, 1::2]], axis=0)  # [128,16,128]
        qhi = Q[b, :, h * SQ : (h + 1) * SQ].astype(np.float16)
        qrep = np.concatenate([qhi, qhi], axis=0)  # [128, SQ]
        # V'^T tiles: vt[p, j, d] = V[b, d, KT*j + p]
        # stored as raw uint16 bits: fp16 for ACT-owned tiles, bf16 for DVE's
        vt16 = np.ascontiguousarray(
            V[b].T.reshape(NKT, KT, D).transpose(1, 0, 2)
        ).astype(np.float16)
        vtb = vt16.astype(ml_dtypes.bfloat16)
        vtmix = np.empty((KT, NKT, D), dtype=np.uint16)
        for j in range(NKT):
            if (j // 2) % NPAIR in ACT_PAIRS:
                vtmix[:, j] = vt16[:, j].view(np.uint16)
            else:
                vtmix[:, j] = vtb[:, j].view(np.uint16)
        maps.append(
            {
                "kst": np.ascontiguousarray(kst.reshape(2 * D, NPAIR * KT)),
                "qrep": np.ascontiguousarray(qrep),
                "vtin": vtmix.reshape(KT, NKT * D),
            }
        )
    return maps


def _run(K, V, Q, trace=False):
    from concourse.bass_utils import run_bass_kernel_spmd

    nc = _build()
    res = run_bass_kernel_spmd(
        nc, _in_maps(K, V, Q), list(range(NCORES)), trace=trace
    )
    Q = np.asarray(Q, dtype=np.float32)
    out = np.empty((B, 2 * D, S), dtype=np.float32)
    out[:, D : 2 * D, :] = Q
    for c in range(NCORES):
        b, h = c // 2, c % 2
        rr = res.results[c]["out_r"].astype(np.float64)
        zz = res.results[c]["out_z"].astype(np.float64)
        num = rr[0:D] + rr[D : 2 * D]
        den = zz.sum(axis=0)
        out[b, 0:D, h * SQ : (h + 1) * SQ] = (num / den[None, :]).astype(np.float32)
    return out, res


def kernel(K, V, Q):
    out, _ = _run(K, V, Q, trace=False)
    return out
